# revision 1
# baseline (speedup 1.0000x reference)
"""GCN layer (message passing) on 8 Trainium2 NeuronCores.

  out = D_in^-1/2 * A^T * D_out^-1/2 * x      (degrees clipped to >= 1)

Strategy (per sharding hint: dst nodes partitioned across cores, edges
sharded by dst so the segment sum is local; src features replicated per
core):
  - 8 cores: core k owns dst nodes [12500k, 12500k+12500).
  - On each core, src nodes split into 8 chunks of 12500; GpSimd group g
    (Q7 core g, partitions 16g..16g+15) holds chunk g's features transposed
    [16 feats x 12501] (col 12500 = zero pad) and gathers its edges with
    ap_gather (d=1: one index pulls a 16-feature column).
  - Edges grouped host-side into per-(dst node, src chunk) segments,
    class-padded so the device segment sum is a fixed-window tensor_reduce.
  - Per-group partials land in class-sorted (permuted) positions; a second
    ap_gather un-permutes to node order; a 128x16 ones matmul on PE sums the
    8 groups; degree normalization on device.

Host does only index-structure marshaling (sort/bucket/pad), dtype casts and
layout transforms; all FP math on features/degrees runs on device.
"""
import numpy as np
from contextlib import ExitStack

N = 100000
D = 16
C = 8               # NeuronCores
G = 8               # src chunks == Q7 groups
NL = N // C         # dst nodes per core (12500)
SC = N // G         # src nodes per chunk (12500)
NE = SC + 1         # table cols per group (+ zero col)
SCK = 8192          # gather chunk: idxs per instruction (per group)
NLP = 12512         # NL padded to %16 for the unpermute gather
CLASSES = [1, 2, 3, 4, 5, 6, 7, 8, 9, 10, 11, 12, 13, 14, 15, 16, 18, 20,
           22, 24, 26, 28, 30, 32, 36, 40, 44, 48, 56, 64, 80, 96, 112, 128,
           192, 256, 384, 512]

_CACHE = {}


def _preprocess(features, src, dst):
    src = np.asarray(src).astype(np.int64)
    dst = np.asarray(dst).astype(np.int64)
    out_deg = np.bincount(src, minlength=N)
    in_deg = np.bincount(dst, minlength=N)
    assert out_deg.max() < 250 and in_deg.max() < 250

    order = np.lexsort((src, dst))
    s_s, d_s = src[order], dst[order]
    core_bounds = np.searchsorted(d_s, np.arange(C + 1) * NL)

    clarr = np.array(CLASSES)
    # ---- per-core segment extraction ------------------------------------
    cores = []
    seg_counts = np.zeros((C, G, len(CLASSES)), dtype=np.int64)
    for k in range(C):
        lo, hi = core_bounds[k], core_bounds[k + 1]
        sk, dk = s_s[lo:hi], (d_s[lo:hi] - k * NL).astype(np.int64)
        gk = sk // SC
        lsk = (sk % SC).astype(np.int16)
        o2 = np.argsort(gk * NL + dk, kind="stable")
        gk, dk, lsk = gk[o2], dk[o2], lsk[o2]
        key = gk * NL + dk
        ukey, ustart, ucnt = np.unique(key, return_index=True,
                                       return_counts=True)
        useg_g = ukey // NL
        useg_n = ukey % NL
        ucls = np.searchsorted(clarr, ucnt)
        for g in range(G):
            seg_counts[k, g] = np.bincount(ucls[useg_g == g],
                                           minlength=len(CLASSES))
        cores.append((useg_g, useg_n, ucls, ustart, ucnt, lsk))

    # ---- fully uniform template (same for all cores AND groups: DVE
    # instructions must start at partition 0, so one reduce covers all
    # 8 groups) + greedy chunk layout ------------------------------------
    tmpl = seg_counts.max(axis=(0, 1))  # [n_classes]
    tmpl[0] += 1                        # leading zero-segment
    base = np.cumsum(tmpl) - tmpl       # seq idx base per class
    nseq = int(tmpl.sum())              # template positions per group
    J = nseq
    assert J < 32768

    seq_pad = np.repeat(clarr, tmpl)    # class-sorted pad sequence
    abs_slot = np.zeros(nseq, dtype=np.int64)
    chunk_ops = []                      # per chunk: [(pad, n, s_off, seg0)]
    chunk_sizes = []
    ptr = 0
    nchunks = 0
    while ptr < nseq:
        ops = []
        fill = 0
        run_pad, run_n, run_off, run_seg0 = 0, 0, 0, 0
        while ptr < nseq and fill + seq_pad[ptr] <= SCK:
            p = int(seq_pad[ptr])
            abs_slot[ptr] = nchunks * SCK + fill
            if p == run_pad and ptr == run_seg0 + run_n:
                run_n += 1
            else:
                if run_n:
                    ops.append((run_pad, run_n, run_off, run_seg0))
                run_pad, run_n, run_off, run_seg0 = p, 1, fill, ptr
            fill += p
            ptr += 1
        if run_n:
            ops.append((run_pad, run_n, run_off, run_seg0))
        chunk_ops.append(ops)
        chunk_sizes.append(fill)
        nchunks += 1
    # truncate the last gather chunk to its used size (rounded to the
    # 16-partition idx wrap); earlier chunks are full by greedy packing
    chunk_sizes = [SCK] * (nchunks - 1) + [-(-chunk_sizes[-1] // 16) * 16]
    LS = (nchunks - 1) * SCK + chunk_sizes[-1]

    # ---- per-core device inputs -----------------------------------------
    f32 = np.asarray(features, dtype=np.float32)
    xT = np.zeros((128, NE), dtype=np.float32)
    odeg = np.ones((128, NE), dtype=np.float32)
    for g in range(G):
        xT[16 * g:16 * g + 16, :SC] = f32[g * SC:(g + 1) * SC, :].T
        odeg[16 * g:16 * g + 16, :SC] = out_deg[g * SC:(g + 1) * SC]
    comb = np.zeros((128, D), dtype=np.float32)
    comb[np.arange(128), np.arange(128) % 16] = 1.0

    in_maps = []
    for k in range(C):
        useg_g, useg_n, ucls, ustart, ucnt, lsk = cores[k]
        slots = np.full((G, LS), SC, dtype=np.int16)
        pos_of_node = np.zeros((G, NLP), dtype=np.int16)
        for g in range(G):
            mg = useg_g == g
            cls_g, n_g = ucls[mg], useg_n[mg]
            start_g, cnt_g = ustart[mg], ucnt[mg]
            # rank within class (segments appear dst-sorted; stable argsort
            # by class gives within-class ordering by dst)
            rank = np.zeros(len(cls_g), dtype=np.int64)
            for ci in np.unique(cls_g):
                m = cls_g == ci
                r0 = 1 if ci == 0 else 0
                rank[m] = r0 + np.arange(m.sum())
            seq_idx = base[cls_g] + rank
            assert seq_idx.max(initial=0) < nseq
            pos_of_node[g, n_g] = seq_idx.astype(np.int16)
            s0 = abs_slot[seq_idx]
            # per-edge slot positions
            tot = int(cnt_g.sum())
            if tot:
                seg_rep = np.repeat(np.arange(len(cnt_g)), cnt_g)
                within = np.arange(tot) - np.repeat(
                    np.cumsum(cnt_g) - cnt_g, cnt_g)
                # edges of these segments: lsk[start..start+cnt) — segments are
                # contiguous slices; build edge value array in segment order
                edge_idx = np.repeat(start_g, cnt_g) + within
                slots[g, s0[seg_rep] + within] = lsk[edge_idx]
        slots_w = slots.reshape(G, LS // 16, 16).transpose(0, 2, 1) \
                       .reshape(128, LS // 16)
        unp_w = pos_of_node.reshape(G, NLP // 16, 16).transpose(0, 2, 1) \
                           .reshape(128, NLP // 16)
        ideg = np.tile(in_deg[k * NL:(k + 1) * NL][None, :].astype(np.float32),
                       (16, 1))
        in_maps.append({"xT": xT, "odeg": odeg, "slots": slots_w,
                        "unp": unp_w, "ideg": ideg, "comb": comb})
    meta = dict(J=J, LS=LS, chunk_ops=chunk_ops, chunk_sizes=chunk_sizes)
    return in_maps, meta


def _build(meta, reps=1):
    import concourse.tile as tile
    from concourse import bacc, mybir

    J, LS, chunk_ops = meta["J"], meta["LS"], meta["chunk_ops"]
    chunk_sizes = meta["chunk_sizes"]
    nc = bacc.Bacc("TRN2", target_bir_lowering=False, debug=False,
                   num_devices=C)
    xT_d = nc.dram_tensor("xT", [128, NE], mybir.dt.float32,
                          kind="ExternalInput").ap()
    odeg_d = nc.dram_tensor("odeg", [128, NE], mybir.dt.float32,
                            kind="ExternalInput").ap()
    slots_d = nc.dram_tensor("slots", [128, LS // 16], mybir.dt.int16,
                             kind="ExternalInput").ap()
    unp_d = nc.dram_tensor("unp", [128, NLP // 16], mybir.dt.int16,
                           kind="ExternalInput").ap()
    ideg_d = nc.dram_tensor("ideg", [16, NL], mybir.dt.float32,
                            kind="ExternalInput").ap()
    comb_d = nc.dram_tensor("comb", [128, D], mybir.dt.float32,
                            kind="ExternalInput").ap()
    out_d = nc.dram_tensor("out", [16, NL], mybir.dt.float32,
                           kind="ExternalOutput").ap()

    with tile.TileContext(nc) as tc, ExitStack() as ctx:
        tabp = ctx.enter_context(tc.tile_pool(name="tab", bufs=1))
        scr = ctx.enter_context(tc.tile_pool(name="scr", bufs=2))
        idxp = ctx.enter_context(tc.tile_pool(name="idx", bufs=2))
        xgp = ctx.enter_context(tc.tile_pool(name="xg", bufs=2))
        prt = ctx.enter_context(tc.tile_pool(name="prt", bufs=1))
        psum = ctx.enter_context(tc.tile_pool(name="ps", bufs=4, space="PSUM"))
        outp = ctx.enter_context(tc.tile_pool(name="outw", bufs=1))

        if reps == 0:
            # null variant for overhead calibration: trivial passthrough
            z = scr.tile([16, NL], mybir.dt.float32, tag="deg")
            nc.sync.dma_start(out=z[:], in_=ideg_d[:])
            nc.sync.dma_start(out=out_d[:], in_=z[:])
        else:
            # P0: load + out-degree-scale the feature table
            xTt = tabp.tile([128, NE], mybir.dt.float32, tag="big")
            nc.sync.dma_start(out=xTt[:], in_=xT_d[:])
            DS = 512
            for c0 in range(0, NE, DS):
                w = min(DS, NE - c0)
                dg = scr.tile([128, DS], mybir.dt.float32, tag="deg")
                nc.sync.dma_start(out=dg[:, :w], in_=odeg_d[:, c0:c0 + w])
                nc.vector.tensor_scalar_max(dg[:, :w], dg[:, :w], 1.0)
                sq = scr.tile([128, DS], mybir.dt.float32, tag="sq")
                nc.scalar.sqrt(sq[:, :w], dg[:, :w])
                nc.vector.reciprocal(dg[:, :w], sq[:, :w])
                nc.vector.tensor_mul(xTt[:, c0:c0 + w], xTt[:, c0:c0 + w],
                                     dg[:, :w])

            # P1: gather + segment-sum (repeated `reps` times for timing
            # variants; passes >1 recompute identical values)
            part = prt.tile([128, J], mybir.dt.float32)
            nc.scalar.memzero(part[:])
            for _ in range(reps):
                for c, CK in enumerate(chunk_sizes):
                    co = c * (SCK // 16)
                    idxt = idxp.tile([128, SCK // 16], mybir.dt.int16)
                    nc.sync.dma_start(out=idxt[:, :CK // 16],
                                      in_=slots_d[:, co:co + CK // 16])
                    xgt = xgp.tile([128, SCK], mybir.dt.float32)
                    nc.gpsimd.ap_gather(xgt[:, :CK], xTt[:], idxt[:, :CK // 16],
                                        channels=128, num_elems=NE, d=1,
                                        num_idxs=CK)
                    for (pad, n, s_off, seg0) in chunk_ops[c]:
                        nc.vector.tensor_reduce(
                            part[:, seg0:seg0 + n],
                            xgt[:, s_off:s_off + n * pad]
                            .rearrange("p (n w) -> p n w", w=pad),
                            axis=mybir.AxisListType.X, op=mybir.AluOpType.add)

            # P2: un-permute partials to node order
            unpt = idxp.tile([128, NLP // 16], mybir.dt.int16, tag="unp")
            nc.sync.dma_start(out=unpt[:], in_=unp_d[:])
            punp = tabp.tile([128, NLP], mybir.dt.float32, tag="big")
            nc.gpsimd.ap_gather(punp[:], part[:], unpt[:],
                                channels=128, num_elems=J, d=1, num_idxs=NLP)

            # P3: combine groups on PE + in-degree normalize + write out
            combt = tabp.tile([128, D], mybir.dt.float32, tag="comb")
            nc.sync.dma_start(out=combt[:], in_=comb_d[:])
            W = 500
            for w0 in range(0, NL, W):
                ps = psum.tile([D, W], mybir.dt.float32)
                nc.tensor.matmul(out=ps[:], lhsT=combt[:],
                                 rhs=punp[:, w0:w0 + W], start=True, stop=True)
                idg = outp.tile([16, W], mybir.dt.float32, tag="idg")
                nc.sync.dma_start(out=idg[:], in_=ideg_d[:, w0:w0 + W])
                nc.vector.tensor_scalar_max(idg[:], idg[:], 1.0)
                sq2 = outp.tile([16, W], mybir.dt.float32, tag="sq2")
                nc.scalar.sqrt(sq2[:], idg[:])
                nc.vector.reciprocal(idg[:], sq2[:])
                ow = outp.tile([16, W], mybir.dt.float32, tag="ow")
                nc.vector.tensor_mul(ow[:], ps[:], idg[:])
                nc.sync.dma_start(out=out_d[:, w0:w0 + W], in_=ow[:])
    nc.compile()
    return nc


def kernel(features, src, dst):
    from concourse.bass_utils import run_bass_kernel_spmd
    in_maps, meta = _preprocess(features, src, dst)
    key = (meta["J"], meta["LS"],
           tuple(tuple(map(tuple, ops)) for ops in meta["chunk_ops"]))
    if key not in _CACHE:
        _CACHE[key] = _build(meta)
    nc = _CACHE[key]
    res = run_bass_kernel_spmd(nc, in_maps, list(range(C)))
    out = np.empty((N, D), dtype=np.float32)
    for k in range(C):
        out[k * NL:(k + 1) * NL, :] = res.results[k]["out"].T
    return out



# revision 7
# speedup vs baseline: 7.5988x; 7.5988x over previous
"""GCN layer (message passing) on 8 Trainium2 NeuronCores.

  out = D_in^-1/2 * A^T * D_out^-1/2 * x      (degrees clipped to >= 1)

Strategy (per sharding hint: dst nodes partitioned across cores, edges
sharded by dst so the segment sum is local; src features replicated per
core):
  - 8 cores: core k owns dst nodes [12500k, 12500k+12500).
  - On each core, src nodes split into 8 chunks of 12500; GpSimd group g
    (Q7 core g, partitions 16g..16g+15) holds chunk g's features transposed
    [16 feats x 12501] (col 12500 = zero pad) and gathers its edges with
    ap_gather (d=1: one index pulls a 16-feature column).
  - Edges grouped host-side into per-(dst node, src chunk) segments,
    class-padded so the device segment sum is a fixed-window tensor_reduce.
  - Per-group partials land in class-sorted (permuted) positions; a second
    ap_gather un-permutes to node order; a 128x16 ones matmul on PE sums the
    8 groups; degree normalization on device.

Host does only index-structure marshaling (sort/bucket/pad), dtype casts and
layout transforms; all FP math on features/degrees runs on device.
"""
import numpy as np
from contextlib import ExitStack

N = 100000
D = 16
C = 8               # NeuronCores
G = 8               # src chunks == Q7 groups
NL = N // C         # dst nodes per core (12500)
SC = N // G         # src nodes per chunk (12500)
NE = SC + 1         # table cols per group (+ zero col)
SCK = 4096          # gather chunk: idxs per instruction (per group)
NLP = 12512         # NL padded to %16 for the unpermute gather
CLASSES = [1, 2, 3, 4, 5, 6, 7, 8, 9, 10, 11, 12, 13, 14, 15, 16, 18, 20,
           22, 24, 26, 28, 30, 32, 36, 40, 44, 48, 56, 64, 80, 96, 112, 128,
           192, 256, 384, 512]

_CACHE = {}


def _preprocess(features, src, dst):
    src = np.asarray(src).astype(np.int64)
    dst = np.asarray(dst).astype(np.int64)
    out_deg = np.bincount(src, minlength=N)
    in_deg = np.bincount(dst, minlength=N)
    assert out_deg.max() < 250 and in_deg.max() < 250

    order = np.lexsort((src, dst))
    s_s, d_s = src[order], dst[order]
    core_bounds = np.searchsorted(d_s, np.arange(C + 1) * NL)

    clarr = np.array(CLASSES)
    # ---- per-core segment extraction ------------------------------------
    cores = []
    seg_counts = np.zeros((C, G, len(CLASSES)), dtype=np.int64)
    for k in range(C):
        lo, hi = core_bounds[k], core_bounds[k + 1]
        sk, dk = s_s[lo:hi], (d_s[lo:hi] - k * NL).astype(np.int64)
        gk = sk // SC
        lsk = (sk % SC).astype(np.int16)
        o2 = np.argsort(gk * NL + dk, kind="stable")
        gk, dk, lsk = gk[o2], dk[o2], lsk[o2]
        key = gk * NL + dk
        ukey, ustart, ucnt = np.unique(key, return_index=True,
                                       return_counts=True)
        useg_g = ukey // NL
        useg_n = ukey % NL
        ucls = np.searchsorted(clarr, ucnt)
        for g in range(G):
            seg_counts[k, g] = np.bincount(ucls[useg_g == g],
                                           minlength=len(CLASSES))
        cores.append((useg_g, useg_n, ucls, ustart, ucnt, lsk))

    # ---- fully uniform template (same for all cores AND groups: DVE
    # instructions must start at partition 0, so one reduce covers all
    # 8 groups) + greedy chunk layout ------------------------------------
    tmpl = seg_counts.max(axis=(0, 1))  # [n_classes]
    tmpl[0] += 1                        # leading zero-segment
    base = np.cumsum(tmpl) - tmpl       # seq idx base per class
    nseq = int(tmpl.sum())              # template positions per group
    J = nseq
    assert J < 32768

    seq_pad = np.repeat(clarr, tmpl)    # class-sorted pad sequence
    abs_slot = np.zeros(nseq, dtype=np.int64)
    chunk_ops = []                      # per chunk: [(pad, n, s_off, seg0)]
    chunk_sizes = []
    ptr = 0
    nchunks = 0
    while ptr < nseq:
        ops = []
        fill = 0
        run_pad, run_n, run_off, run_seg0 = 0, 0, 0, 0
        while ptr < nseq and fill + seq_pad[ptr] <= SCK:
            p = int(seq_pad[ptr])
            abs_slot[ptr] = nchunks * SCK + fill
            if p == run_pad and ptr == run_seg0 + run_n:
                run_n += 1
            else:
                if run_n:
                    ops.append((run_pad, run_n, run_off, run_seg0))
                run_pad, run_n, run_off, run_seg0 = p, 1, fill, ptr
            fill += p
            ptr += 1
        if run_n:
            ops.append((run_pad, run_n, run_off, run_seg0))
        chunk_ops.append(ops)
        chunk_sizes.append(fill)
        nchunks += 1
    # truncate the last gather chunk to its used size (rounded to the
    # 16-partition idx wrap); earlier chunks are full by greedy packing
    chunk_sizes = [SCK] * (nchunks - 1) + [-(-chunk_sizes[-1] // 16) * 16]
    LS = (nchunks - 1) * SCK + chunk_sizes[-1]

    # ---- per-core device inputs -----------------------------------------
    f32 = np.asarray(features, dtype=np.float32)
    xT = np.zeros((128, NE), dtype=np.float32)
    odeg = np.ones((128, NE), dtype=np.float32)
    for g in range(G):
        xT[16 * g:16 * g + 16, :SC] = f32[g * SC:(g + 1) * SC, :].T
        odeg[16 * g:16 * g + 16, :SC] = out_deg[g * SC:(g + 1) * SC]
    comb = np.zeros((128, D), dtype=np.float32)
    comb[np.arange(128), np.arange(128) % 16] = 1.0

    in_maps = []
    for k in range(C):
        useg_g, useg_n, ucls, ustart, ucnt, lsk = cores[k]
        slots = np.full((G, LS), SC, dtype=np.int16)
        pos_of_node = np.zeros((G, NLP), dtype=np.int16)
        for g in range(G):
            mg = useg_g == g
            cls_g, n_g = ucls[mg], useg_n[mg]
            start_g, cnt_g = ustart[mg], ucnt[mg]
            # rank within class (segments appear dst-sorted; stable argsort
            # by class gives within-class ordering by dst)
            rank = np.zeros(len(cls_g), dtype=np.int64)
            for ci in np.unique(cls_g):
                m = cls_g == ci
                r0 = 1 if ci == 0 else 0
                rank[m] = r0 + np.arange(m.sum())
            seq_idx = base[cls_g] + rank
            assert seq_idx.max(initial=0) < nseq
            pos_of_node[g, n_g] = seq_idx.astype(np.int16)
            s0 = abs_slot[seq_idx]
            # per-edge slot positions
            tot = int(cnt_g.sum())
            if tot:
                seg_rep = np.repeat(np.arange(len(cnt_g)), cnt_g)
                within = np.arange(tot) - np.repeat(
                    np.cumsum(cnt_g) - cnt_g, cnt_g)
                # edges of these segments: lsk[start..start+cnt) — segments are
                # contiguous slices; build edge value array in segment order
                edge_idx = np.repeat(start_g, cnt_g) + within
                slots[g, s0[seg_rep] + within] = lsk[edge_idx]
        slots_w = slots.reshape(G, LS // 16, 16).transpose(0, 2, 1) \
                       .reshape(128, LS // 16)
        unp_w = pos_of_node.reshape(G, NLP // 16, 16).transpose(0, 2, 1) \
                           .reshape(128, NLP // 16)
        ideg = np.tile(in_deg[k * NL:(k + 1) * NL][None, :].astype(np.float32),
                       (16, 1))
        in_maps.append({"xT": xT, "odeg": odeg, "slots": slots_w,
                        "unp": unp_w, "ideg": ideg, "comb": comb})
    meta = dict(J=J, LS=LS, chunk_ops=chunk_ops, chunk_sizes=chunk_sizes)
    return in_maps, meta


def _build(meta, reps=1, tail_reps=1):
    import concourse.tile as tile
    from concourse import bacc, mybir

    J, LS, chunk_ops = meta["J"], meta["LS"], meta["chunk_ops"]
    chunk_sizes = meta["chunk_sizes"]
    nc = bacc.Bacc("TRN2", target_bir_lowering=False, debug=False,
                   num_devices=C)
    xT_d = nc.dram_tensor("xT", [128, NE], mybir.dt.float32,
                          kind="ExternalInput").ap()
    odeg_d = nc.dram_tensor("odeg", [128, NE], mybir.dt.float32,
                            kind="ExternalInput").ap()
    slots_d = nc.dram_tensor("slots", [128, LS // 16], mybir.dt.int16,
                             kind="ExternalInput").ap()
    unp_d = nc.dram_tensor("unp", [128, NLP // 16], mybir.dt.int16,
                           kind="ExternalInput").ap()
    ideg_d = nc.dram_tensor("ideg", [16, NL], mybir.dt.float32,
                            kind="ExternalInput").ap()
    comb_d = nc.dram_tensor("comb", [128, D], mybir.dt.float32,
                            kind="ExternalInput").ap()
    out_d = nc.dram_tensor("out", [16, NL], mybir.dt.float32,
                           kind="ExternalOutput").ap()

    with tile.TileContext(nc) as tc, ExitStack() as ctx:
        tabp = ctx.enter_context(tc.tile_pool(name="tab", bufs=1))
        scr = ctx.enter_context(tc.tile_pool(name="scr", bufs=2))
        idxp = ctx.enter_context(tc.tile_pool(name="idx", bufs=2))
        xgp = ctx.enter_context(tc.tile_pool(name="xg", bufs=2))
        prt = ctx.enter_context(tc.tile_pool(name="prt", bufs=1))
        psum = ctx.enter_context(tc.tile_pool(name="ps", bufs=4, space="PSUM"))
        outp = ctx.enter_context(tc.tile_pool(name="outw", bufs=1))

        if reps == 0:
            # null variant for overhead calibration: trivial passthrough
            z = scr.tile([16, NL], mybir.dt.float32, tag="deg")
            nc.sync.dma_start(out=z[:], in_=ideg_d[:])
            nc.sync.dma_start(out=out_d[:], in_=z[:])
        else:
            # P0: load + out-degree-scale the feature table in 4 wide
            # chunks (short pre-gather critical path, small scratch).
            xTt = tabp.tile([128, NE], mybir.dt.float32, tag="big")
            nc.sync.dma_start(out=xTt[:], in_=xT_d[:])
            DS = 2112
            for c0 in range(0, NE, DS):
                w = min(DS, NE - c0)
                dg = scr.tile([128, DS], mybir.dt.float32, tag="deg")
                nc.sync.dma_start(out=dg[:, :w], in_=odeg_d[:, c0:c0 + w])
                nc.vector.tensor_scalar_max(dg[:, :w], dg[:, :w], 1.0)
                nc.scalar.sqrt(dg[:, :w], dg[:, :w])
                nc.vector.reciprocal(dg[:, :w], dg[:, :w])
                nc.vector.tensor_mul(xTt[:, c0:c0 + w], xTt[:, c0:c0 + w],
                                     dg[:, :w])

            # in-degree rsqrt computed wide, directly in the output tile
            # (scaled in-place in P3); independent of the gather stream,
            # so it runs underneath it, off the critical path.
            ow = outp.tile([16, NL], mybir.dt.float32, tag="ow")
            nc.sync.dma_start(out=ow[:], in_=ideg_d[:])
            nc.vector.tensor_scalar_max(ow[:], ow[:], 1.0)
            nc.scalar.sqrt(ow[:], ow[:])
            nc.vector.reciprocal(ow[:], ow[:])
            combt = tabp.tile([128, D], mybir.dt.float32, tag="comb")
            nc.sync.dma_start(out=combt[:], in_=comb_d[:])

            part = prt.tile([128, J], mybir.dt.float32)
            nc.scalar.memzero(part[:])

            # P1: gather + segment-sum (repeated `reps` times for timing
            # variants; passes >1 recompute identical values)
            for _ in range(reps):
                for c, CK in enumerate(chunk_sizes):
                    co = c * (SCK // 16)
                    idxt = idxp.tile([128, SCK // 16], mybir.dt.int16)
                    nc.sync.dma_start(out=idxt[:, :CK // 16],
                                      in_=slots_d[:, co:co + CK // 16])
                    xgt = xgp.tile([128, SCK], mybir.dt.float32)
                    nc.gpsimd.ap_gather(xgt[:, :CK], xTt[:], idxt[:, :CK // 16],
                                        channels=128, num_elems=NE, d=1,
                                        num_idxs=CK)
                    for (pad, n, s_off, seg0) in chunk_ops[c]:
                        nc.vector.tensor_reduce(
                            part[:, seg0:seg0 + n],
                            xgt[:, s_off:s_off + n * pad]
                            .rearrange("p (n w) -> p n w", w=pad),
                            axis=mybir.AxisListType.X, op=mybir.AluOpType.add)

            # P2+P3 (repeated `tail_reps` times for timing variants):
            # un-permute partials to node order, combine groups on PE,
            # normalize by precomputed ideg^-1/2, single output DMA.
            for _ in range(tail_reps):
                unpt = idxp.tile([128, NLP // 16], mybir.dt.int16, tag="unp")
                nc.sync.dma_start(out=unpt[:], in_=unp_d[:])
                punp = tabp.tile([128, NLP], mybir.dt.float32, tag="big")
                nc.gpsimd.ap_gather(punp[:], part[:], unpt[:],
                                    channels=128, num_elems=J, d=1,
                                    num_idxs=NLP)
                W = 500
                for w0 in range(0, NL, W):
                    ps = psum.tile([D, W], mybir.dt.float32)
                    nc.tensor.matmul(out=ps[:], lhsT=combt[:],
                                     rhs=punp[:, w0:w0 + W], start=True,
                                     stop=True)
                    # scale in place: ow holds ideg^-1/2 until overwritten
                    nc.vector.tensor_mul(ow[:, w0:w0 + W], ps[:],
                                         ow[:, w0:w0 + W])
            nc.sync.dma_start(out=out_d[:], in_=ow[:])
    nc.compile()
    return nc


def kernel(features, src, dst):
    from concourse.bass_utils import run_bass_kernel_spmd
    in_maps, meta = _preprocess(features, src, dst)
    key = (meta["J"], meta["LS"],
           tuple(tuple(map(tuple, ops)) for ops in meta["chunk_ops"]))
    if key not in _CACHE:
        _CACHE[key] = _build(meta)
    nc = _CACHE[key]
    res = run_bass_kernel_spmd(nc, in_maps, list(range(C)))
    out = np.empty((N, D), dtype=np.float32)
    for k in range(C):
        out[k * NL:(k + 1) * NL, :] = res.results[k]["out"].T
    return out

